# revision 41
# baseline (speedup 1.0000x reference)
"""Trainium2 Bass kernel for nn_Attention_7911329759504 (GQA attention,
B=1, S=2048, H=2048, 32 query heads / 8 KV heads, head_dim 64, RoPE,
causal mask, fp32 in/out).

Strategy: tensor-parallel across 8 NeuronCores by KV head -- each core owns
one KV head and its 4 query heads (shards Wqkv rows / Wo columns by head),
computes a full partial output, and the host sums the 8 partials (the
"all-reduce after wo" done on the host since each core's output is a pure
summand).

All matmul operands are bf16 (PSUM accumulation fp32): enables the PE
fast-weight-load path (fp32r serialized every LDWEIGHTS ~184ns against its
matmul), halves HBM traffic, and doubles DVE throughput on elementwise ops.

PSUM layout (8 banks of [128 x 512 fp32]):
  m0, m1       -- qkv accumulators (j0/j1 pass, then j2 pass reusing m0),
                  rope swap, v-transpose, norm broadcast, wo outputs
  sc x2 buffers -- score tiles [128, 2, SCH], double-buffered so entry
                  e+1's score matmuls don't stall on exp(e) reading sct
  pvA, pvB     -- attention pv accumulators (one per head of the jo pair)

Self-contained: hardcodes all shapes; only imports concourse from the
system install. `kernel(**inputs)` takes the full unsharded inputs and
returns the full [1, S, H] float32 output.
"""

import sys

sys.path.insert(0, "/opt/trn_rl_repo")

import numpy as np
import ml_dtypes

import concourse.bass as bass
import concourse.mybir as mybir
import concourse.tile as tile

F32 = mybir.dt.float32
BF16 = mybir.dt.bfloat16
AF = mybir.ActivationFunctionType
ALU = mybir.AluOpType
BF_NP = ml_dtypes.bfloat16

S = 2048
H = 2048
NH, NKV, HD = 32, 8, 64
G = NH // NKV            # query heads per kv head = 4
JL = G * HD + 2 * HD     # local qkv rows per core = 384
YL = G * HD              # local y rows per core = 256
SCH = 512                # s-chunk (psum bank width in fp32)
NCH = S // SCH           # 4 s-chunks
NKT = S // 128           # 16 t-tiles
NTILES_H = H // 128      # 16 contraction tiles for the projections
N_CORES = 8

MAX_RESIDENT_MASKS = 5


def make_schedule(mask_np):
    """Per (s-chunk, t-tile) status from the actual [S, S] bool mask.

    Returns (sched, mask_tiles, band_mode):
      sched[chunk] = list of (ti, mask_spec or None); skipped tiles omitted.
      mask_tiles: None (band mode / no partials) or [n, 128, SCH] f32 array.
      band_mode: True when mask is exactly tril (use the shared band const).
    """
    tril = np.tril(np.ones((S, S), dtype=bool))
    band_mode = np.array_equal(mask_np, tril)
    sched = []
    tiles = []
    for c in range(NCH):
        s0 = c * SCH
        entries = []
        for ti in range(NKT):
            t0 = ti * 128
            blk = mask_np[s0 : s0 + SCH, t0 : t0 + 128]  # [s, t]
            if not blk.any():
                continue
            if blk.all():
                entries.append((ti, None))
            elif band_mode:
                # partial tile of tril: band slice at offset 384 - (t0 - s0)
                entries.append((ti, ("band", 384 - (t0 - s0))))
            else:
                tiles.append(blk.T.astype(np.float32))  # [t(128), s(SCH)]
                entries.append((ti, ("gen", len(tiles) - 1)))
        sched.append(entries)
    mask_tiles = np.stack(tiles) if tiles else None
    return sched, mask_tiles, band_mode


def build_nc(sched, n_gen_masks, band_mode):
    nc = bass.Bass(target_bir_lowering=False)

    # p-major host layouts: every DMA descriptor row is 4KB+ contiguous.
    # The DMA subsystem is packet-rate-bound (~150ns per descriptor row),
    # so [H, S]-style layouts (1KB rows) kept the queues busy end-to-end.
    xT = nc.declare_dram_parameter("xT", [128, H // 128, S], BF16, isOutput=False)
    wqkvT = nc.declare_dram_parameter(
        "wqkvT", [128, (H // 128) * JL], BF16, isOutput=False
    )
    woT = nc.declare_dram_parameter("woT", [128, 2, H], BF16, isOutput=False)
    ctab = nc.declare_dram_parameter("ctab", [128, S], BF16, isOutput=False)
    stab = nc.declare_dram_parameter("stab", [128, S], BF16, isOutput=False)
    consts = nc.declare_dram_parameter("consts", [128, 768], BF16, isOutput=False)
    # consts columns: [0:128] pswap, [128:256] identity, [384:448] ones block,
    # [512:640] sel0, [640:768] sel1 (denominator row-broadcast selectors)
    band = None
    if band_mode:
        band = nc.declare_dram_parameter("band", [128, 896], BF16, isOutput=False)
    gmask = None
    if n_gen_masks:
        gmask = nc.declare_dram_parameter(
            "gmask", [n_gen_masks, 128, SCH], BF16, isOutput=False
        )
    out_t = nc.declare_dram_parameter("out_t", [H, S], BF16, isOutput=True)

    resident_masks = bool(n_gen_masks) and n_gen_masks <= MAX_RESIDENT_MASKS
    NTILES = H // 128

    with tile.TileContext(nc) as tc:
        with (
            tc.tile_pool(name="const", bufs=1) as cpool,
            tc.tile_pool(name="psb", bufs=6) as p_pool,
            tc.tile_pool(name="tmp", bufs=2) as tmp_pool,
            tc.tile_pool(name="osb", bufs=6) as o_pool,
        ):
            # ---- persistent SBUF tensors ----
            x_sb = cpool.tile([128, NTILES, S], BF16, tag="x")
            wq_sb = cpool.tile([128, NTILES, JL], BF16, tag="wq")
            wo_sb = cpool.tile([128, 2, H], BF16, tag="wo")
            c_sb = cpool.tile([128, S], BF16, tag="ctab")
            s_sb = cpool.tile([128, S], BF16, tag="stab")
            k_sb = cpool.tile([128, 768], BF16, tag="consts")
            qkv_sb = cpool.tile([128, 3, S], BF16, tag="qkv")
            # zero-padded roped-k copies: _lo has k in rows 0:64 (even heads),
            # _hi in rows 64:128 (odd heads); opposite halves zero so score
            # matmuls run with full K=128 geometry. (2x-row-tiled K=64 score
            # pairs were tried and REGRESSED ~25%: the mode switches against
            # the 128-mode pv/filler matmuls drain the PE pipeline.)
            kdup_lo = cpool.tile([128, S], BF16, tag="kdlo")
            kdup_hi = cpool.tile([128, S], BF16, tag="kdhi")
            # v tiles padded to 128 stationary columns so FWL stays enabled
            # (cols 0:64 = v dims, col 64 = ones for the denominator row,
            # cols 65:128 zeroed once at startup)
            v_sb = cpool.tile([128, NKT, 128], BF16, tag="vt")
            y_sb = cpool.tile([128, 2, S], BF16, tag="yt")
            den_sb = cpool.tile([128, S], F32, tag="den")
            rec_sb = cpool.tile([128, S], BF16, tag="rec")
            band_sb = None
            if band_mode:
                band_sb = cpool.tile([128, 896], BF16, tag="band")
            gm_sb = None
            if resident_masks:
                gm_sb = cpool.tile([128, n_gen_masks, SCH], BF16, tag="gm")

            pswap = k_sb[:, 0:128]
            ident = k_sb[:, 128:256]

            def emit_wq_group(g):
                nc.sync.dma_start(
                    out=wq_sb[:, 4 * g : 4 * g + 4, :],
                    in_=wqkvT[:, g * 4 * JL : (g + 1) * 4 * JL],
                )

            with (
                tc.tile_pool(name="mainps", bufs=1, space="PSUM") as mps,
                tc.tile_pool(name="scps", bufs=2, space="PSUM") as sc_psum,
                tc.tile_pool(name="pvps", bufs=1, space="PSUM") as pv_psum,
            ):

                def emit_qkv_stepA(ch, k, ps01):
                    cs = slice(ch * SCH, (ch + 1) * SCH)
                    if ch == 0 and k % 4 == 0:
                        emit_wq_group(k // 4)
                    nc.sync.dma_start(out=x_sb[:, k, cs], in_=xT[:, k, cs])
                    for j in range(2):
                        nc.tensor.matmul(
                            ps01[j],
                            wq_sb[:, k, j * 128 : (j + 1) * 128],
                            x_sb[:, k, cs],
                            start=(k == 0),
                            stop=(k == NTILES - 1),
                        )

                def emit_qkv_stepB(ch, k, ps2):
                    cs = slice(ch * SCH, (ch + 1) * SCH)
                    nc.tensor.matmul(
                        ps2[0],
                        wq_sb[:, k, 256:384],
                        x_sb[:, k, cs],
                        start=(k == 0),
                        stop=(k == NTILES - 1),
                    )

                def emit_qkv_copyback(ch, j, ps):
                    cs = slice(ch * SCH, (ch + 1) * SCH)
                    nc.vector.tensor_copy(qkv_sb[:, j, cs], ps)

                def make_qkv_filler(ch):
                    ps0 = mps.tile([128, SCH], F32, tag="m0", name="qkvps0")
                    ps1 = mps.tile([128, SCH], F32, tag="m1", name="qkvps1")
                    items = [
                        (lambda k=k: emit_qkv_stepA(ch, k, (ps0, ps1)))
                        for k in range(NTILES)
                    ]
                    items.append(lambda: emit_qkv_copyback(ch, 0, ps0))
                    items.append(lambda: emit_qkv_copyback(ch, 1, ps1))
                    ps2 = []

                    def start_b():
                        ps2.append(mps.tile([128, SCH], F32, tag="m0", name="qkvps2"))

                    items.append(start_b)
                    items += [
                        (lambda k=k: emit_qkv_stepB(ch, k, ps2))
                        for k in range(NTILES)
                    ]
                    items.append(lambda: emit_qkv_copyback(ch, 2, ps2[0]))
                    return items

                def emit_wo_step(ch, ot, slots):
                    cs = slice(ch * SCH, (ch + 1) * SCH)
                    os_ = slice(ot * 128, (ot + 1) * 128)
                    wp = slots[ot % len(slots)]()
                    for jo in range(2):
                        nc.tensor.matmul(
                            wp[:],
                            wo_sb[:, jo, os_],
                            y_sb[:, jo, cs],
                            start=(jo == 0),
                            stop=(jo == 1),
                        )
                    ob = o_pool.tile([128, SCH], BF16, tag="ob", name="ob")
                    if ch == NCH - 1:
                        # tail: halve the psum-evacuation latency by copying
                        # each half on a different engine (both are idle-ish)
                        nc.scalar.copy(ob[:, 0:256], wp[:, 0:256])
                        nc.vector.tensor_copy(ob[:, 256:512], wp[:, 256:512])
                    elif ot % 2 == 0:
                        nc.scalar.copy(ob[:], wp[:])
                    else:
                        nc.vector.tensor_copy(ob[:], wp[:])
                    # tail stores split across both hwdge queues (ACT is idle
                    # by then); earlier chunks stay on sync so DMA issues
                    # never delay the exp stream on the scalar engine
                    if ch == NCH - 1 and ot % 2 == 1:
                        nc.scalar.dma_start(out=out_t[os_, cs], in_=ob[:])
                    else:
                        nc.sync.dma_start(out=out_t[os_, cs], in_=ob[:])

                def wo_slots_m01():
                    return [
                        lambda: mps.tile([128, SCH], F32, tag="m0", name="wops"),
                        lambda: mps.tile([128, SCH], F32, tag="m1", name="wops"),
                    ]

                def make_wo_filler(ch, slots=None):
                    if slots is None:
                        slots = wo_slots_m01()
                    return [
                        (lambda ot=ot: emit_wo_step(ch, ot, slots))
                        for ot in range(H // 128)
                    ]

                def emit_rope_jo(ch, jo):
                    cs = slice(ch * SCH, (ch + 1) * SCH)
                    pcount = 128 if jo < 2 else 64
                    swt = mps.tile([128, SCH], F32, tag="m1", name="swps")
                    # full 128-col pswap keeps NumWeights==128 (FWL); for
                    # jo=2 rows 64:128 of swt are swapped-v garbage, unread
                    nc.tensor.matmul(
                        swt[:],
                        pswap,
                        qkv_sb[:, jo, cs],
                        start=True,
                        stop=True,
                    )
                    t0 = tmp_pool.tile([128, SCH], BF16, tag="ropet0")
                    nc.vector.tensor_mul(
                        t0[:pcount], qkv_sb[:pcount, jo, cs], c_sb[:pcount, cs]
                    )
                    t1 = tmp_pool.tile([128, SCH], BF16, tag="ropet1")
                    nc.vector.tensor_mul(
                        t1[:pcount], swt[:pcount], s_sb[:pcount, cs]
                    )
                    nc.vector.tensor_add(
                        qkv_sb[:pcount, jo, cs], t0[:pcount], t1[:pcount]
                    )

                def emit_rope_kv(ch):
                    cs = slice(ch * SCH, (ch + 1) * SCH)
                    # roped k into the zero-padded lo/hi copies
                    nc.vector.tensor_copy(kdup_lo[0:64, cs], qkv_sb[0:64, 2, cs])
                    nc.vector.tensor_copy(kdup_hi[64:128, cs], qkv_sb[0:64, 2, cs])
                    # v transpose for this chunk's t-tiles
                    for kt in range(4 * ch, 4 * ch + 4):
                        tp = mps.tile([128, 64], BF16, tag="m0", name="vtps")
                        nc.tensor.transpose(
                            tp,
                            qkv_sb[64:128, 2, kt * 128 : (kt + 1) * 128],
                            ident[64:128, 64:128],
                        )
                        nc.vector.tensor_copy(v_sb[:, kt, 0:64], tp)

                def emit_rope(ch):
                    for jo in range(3):
                        emit_rope_jo(ch, jo)
                    emit_rope_kv(ch)

                def make_rope_filler(ch):
                    items = [
                        (lambda jo=jo: emit_rope_jo(ch, jo)) for jo in range(3)
                    ]
                    items.append(lambda: emit_rope_kv(ch))
                    return items

                def emit_const_dmas():
                    # issued from the (idle at startup) ACT queue so they
                    # don't delay the first xt/wq loads on the sync queue
                    nc.scalar.dma_start(out=c_sb[:], in_=ctab[:])
                    nc.scalar.dma_start(out=s_sb[:], in_=stab[:])
                    nc.scalar.dma_start(out=k_sb[:], in_=consts[:])
                    if band_mode:
                        nc.scalar.dma_start(out=band_sb[:], in_=band[:])
                    if resident_masks:
                        nc.scalar.dma_start(
                            out=gm_sb[:], in_=gmask.rearrange("n p f -> p n f")
                        )
                    nc.scalar.dma_start(out=wo_sb[:], in_=woT[:])

                def emit_const_init():
                    # den_sb := 1.0, rec_sb := 0 (garbage rows must stay
                    # finite: sel matmuls read the full rec partition range)
                    nc.vector.tensor_scalar(
                        den_sb[:], c_sb[:], 0.0, 1.0, ALU.mult, ALU.add
                    )
                    nc.vector.tensor_scalar(
                        rec_sb[:], c_sb[:], 0.0, 0.0, ALU.mult, ALU.mult
                    )
                    # ones column of v_hat; zero pad cols; zero halves of the
                    # k copies
                    nc.vector.tensor_copy(v_sb[:, :, 64], k_sb[:, 384 : 384 + NKT])
                    for kt in range(NKT):
                        nc.vector.tensor_scalar(
                            v_sb[:, kt, 65:128], c_sb[:, 0:63], 0.0, 0.0,
                            ALU.mult, ALU.mult,
                        )
                    nc.scalar.activation(
                        kdup_lo[64:128, :], c_sb[64:128, :], AF.Copy, scale=0.0
                    )
                    nc.scalar.activation(
                        kdup_hi[0:64, :], c_sb[0:64, :], AF.Copy, scale=0.0
                    )

                def emit_attn(ch, filler, filler_jo1=()):
                    """Attention for chunk ch; filler items are interleaved
                    into the t-loop to keep the PE fed while exp runs.
                    filler_jo1 items are only drained during the jo=1 pass
                    (they depend on jo=0's outputs)."""
                    cs = slice(ch * SCH, (ch + 1) * SCH)
                    entries = sched[ch]
                    queue = list(filler)
                    n_iters = max(2 * len(entries), 1)
                    per_iter = -(-(len(queue) + len(filler_jo1)) // n_iters)
                    state = {"idx": 0}

                    def drain_filler(n):
                        for _ in range(n):
                            if state["idx"] < len(queue):
                                queue[state["idx"]]()
                                state["idx"] += 1

                    for jo in range(2):
                        if not entries:
                            continue
                        if jo == 1:
                            queue.extend(filler_jo1)
                        pvA = pv_psum.tile([128, SCH], F32, tag="pvA", name="pvA")
                        pvB = pv_psum.tile([128, SCH], F32, tag="pvB", name="pvB")
                        pvs = (pvA, pvB)
                        pending = []

                        def flush_pending(keep):
                            while len(pending) > keep:
                                pp, pei = pending.pop(0)
                                for hp in range(2):
                                    nc.tensor.matmul(
                                        pvs[hp][:],
                                        v_sb[:, entries[pei][0], :],
                                        pp[:, hp, :],
                                        start=(pei == 0),
                                        stop=(pei == len(entries) - 1),
                                    )

                        for ei, (ti, mk) in enumerate(entries):
                            sct = sc_psum.tile(
                                [128, 2, SCH], F32, tag="sc", name="sc"
                            )
                            tc_sl = slice(ti * 128, (ti + 1) * 128)
                            nc.tensor.matmul(
                                sct[:, 0, :], kdup_lo[:, tc_sl],
                                qkv_sb[:, jo, cs], start=True, stop=True,
                            )
                            nc.tensor.matmul(
                                sct[:, 1, :], kdup_hi[:, tc_sl],
                                qkv_sb[:, jo, cs], start=True, stop=True,
                            )
                            p_big = p_pool.tile(
                                [128, 2, SCH], BF16, tag="p", name="p"
                            )
                            nc.scalar.activation(
                                p_big[:], sct[:], AF.Exp, scale=0.125
                            )
                            if mk is not None:
                                kind, arg = mk
                                for hp in range(2):
                                    if kind == "band":
                                        nc.vector.tensor_mul(
                                            p_big[:, hp, :], p_big[:, hp, :],
                                            band_sb[:, arg : arg + SCH],
                                        )
                                    elif resident_masks:
                                        nc.vector.tensor_mul(
                                            p_big[:, hp, :], p_big[:, hp, :],
                                            gm_sb[:, arg, :],
                                        )
                                    else:
                                        mt = tmp_pool.tile(
                                            [128, SCH], BF16, tag="mstream"
                                        )
                                        nc.sync.dma_start(
                                            out=mt[:], in_=gmask[arg]
                                        )
                                        nc.vector.tensor_mul(
                                            p_big[:, hp, :], p_big[:, hp, :],
                                            mt[:],
                                        )
                            pending.append((p_big, ei))
                            flush_pending(2)
                            drain_filler(per_iter)
                        flush_pending(0)
                        # unnormalized y (cross-base for odd heads) + den rows
                        for hp in range(2):
                            h = 2 * jo + hp
                            bp = hp * 64
                            nc.vector.tensor_copy(
                                y_sb[bp : bp + 64, jo, cs], pvs[hp][0:64]
                            )
                            nc.vector.tensor_copy(
                                den_sb[32 * h : 32 * h + 1, cs], pvs[hp][64:65]
                            )
                    drain_filler(len(queue))

                def emit_norm_jo(ch, jo):
                    cs = slice(ch * SCH, (ch + 1) * SCH)
                    rs = slice(64 * jo, 64 * jo + 64)
                    # 1/x = exp(-ln(x)): DVE reciprocal on few partitions is
                    # pathologically slow; ACT ln+exp is flat-rate
                    nc.scalar.activation(den_sb[rs, cs], den_sb[rs, cs], AF.Ln)
                    nc.scalar.activation(
                        rec_sb[rs, cs], den_sb[rs, cs], AF.Exp, scale=-1.0
                    )
                    sel = k_sb[:, 512 + 128 * jo : 640 + 128 * jo]
                    bct = mps.tile([128, SCH], F32, tag="m1", name="bcps")
                    nc.tensor.matmul(
                        bct[:], sel, rec_sb[:, cs], start=True, stop=True
                    )
                    nc.vector.tensor_mul(y_sb[:, jo, cs], y_sb[:, jo, cs], bct[:])

                def make_norm_filler(ch):
                    return [
                        lambda: emit_norm_jo(ch, 0),
                        lambda: emit_norm_jo(ch, 1),
                    ]

                # ---- prologue: qkv(0) + consts + rope(0); rope of the q
                # pairs overlaps the j2 (kv) accumulation pass, whose psum
                # slot (m0 gen2) is disjoint from rope's swt slot (m1) ----
                emit_const_dmas()
                q0 = make_qkv_filler(0)
                for item in q0[:20]:  # 16 stepA + cb0 + cb1 + startB + stepB k0
                    item()
                emit_const_init()
                for item in q0[20:24]:
                    item()
                emit_rope_jo(0, 0)
                for item in q0[24:28]:
                    item()
                emit_rope_jo(0, 1)
                for item in q0[28:]:
                    item()
                emit_rope_jo(0, 2)
                emit_rope_kv(0)

                # ---- main loop: attn(c) with later qkv and wo(c-1) woven
                # in; wo chunks spread over attn(2) and attn(3) so the out
                # stores stream early instead of piling up at the end ----
                for c in range(NCH):
                    filler = []
                    filler_jo1 = ()
                    if 1 <= c <= 2:
                        filler += make_norm_filler(c - 1)
                    if c + 1 < NCH:
                        filler += make_qkv_filler(c + 1)
                        filler += make_rope_filler(c + 1)
                        if c == 2:
                            filler += make_wo_filler(0)
                    else:
                        filler += make_norm_filler(c - 1)
                        filler += make_wo_filler(1)
                        filler += make_wo_filler(2)
                        filler_jo1 = [lambda: emit_norm_jo(NCH - 1, 0)]
                    emit_attn(c, filler, filler_jo1)
                    if c == NCH - 1:
                        emit_norm_jo(c, 1)

                # ---- tail: wo(3) across m0/m1 + both sc buffers' halves ----
                tail_sct1 = sc_psum.tile([128, 2, SCH], F32, tag="sc", name="sc")
                tail_sct2 = sc_psum.tile([128, 2, SCH], F32, tag="sc", name="sc")
                tail_slots = wo_slots_m01() + [
                    (lambda: tail_sct1[:, 0, :]),
                    (lambda: tail_sct1[:, 1, :]),
                    (lambda: tail_sct2[:, 0, :]),
                    (lambda: tail_sct2[:, 1, :]),
                ]
                for item in make_wo_filler(NCH - 1, slots=tail_slots):
                    item()

    fixup_multi_waits(nc)
    return nc


def fixup_multi_waits(nc):
    """walrus CoreV2/V3 codegen rejects instructions carrying more than one
    sync wait. Split extra waits onto same-engine NoOps inserted before."""
    n_split = 0
    for fn in nc.m.functions:
        for bb in fn.blocks:
            new_insts = []
            for inst in bb.instructions:
                si = inst.sync_info
                if si is not None and si.on_wait and len(si.on_wait) > 1:
                    waits = list(si.on_wait)
                    for w in waits[:-1]:
                        n_split += 1
                        nop = mybir.InstNoOp(
                            name=f"I-waitsplit-{n_split}",
                            engine=inst.engine,
                            ins=[],
                            outs=[],
                            sync_info=mybir.SyncInfo(on_wait=[w], on_update=[]),
                        )
                        new_insts.append(nop)
                    si.on_wait = [waits[-1]]
                new_insts.append(inst)
            bb.instructions[:] = new_insts
    return n_split


def host_prep(x, freqs_cis, mask, Wqkv, Wo):
    """Build per-core input maps + the shared schedule."""
    x = np.asarray(x, dtype=np.float32)
    freqs_cis = np.asarray(freqs_cis, dtype=np.float32)
    mask_np = np.asarray(mask).reshape(S, S).astype(bool)
    Wqkv = np.asarray(Wqkv, dtype=np.float32)
    Wo = np.asarray(Wo, dtype=np.float32)

    sched, mask_tiles, band_mode = make_schedule(mask_np)

    # p-major: xT[p, k, s] = x[s, k*128 + p]; per-partition rows contiguous
    xT = np.ascontiguousarray(
        x.reshape(S, H).T.reshape(NTILES_H, 128, S).transpose(1, 0, 2).astype(BF_NP)
    )

    cos_t = np.ascontiguousarray(freqs_cis[:, :, 0].T)  # [32, S]
    sin_t = np.ascontiguousarray(freqs_cis[:, :, 1].T)
    c64 = np.repeat(cos_t, 2, axis=0)  # [64, S]
    s64 = np.repeat(sin_t, 2, axis=0)
    ctab = np.tile(c64, (2, 1)).astype(BF_NP)  # [128, S]
    stab = np.tile(s64, (2, 1)).astype(BF_NP)

    # pswap: out[m] = -in[m+1] (m even), +in[m-1] (m odd); lhsT[k, m]
    pswap = np.zeros((128, 128), dtype=np.float32)
    for i in range(64):
        pswap[2 * i + 1, 2 * i] = -1.0
        pswap[2 * i, 2 * i + 1] = 1.0
    consts = np.zeros((128, 768), dtype=np.float32)
    consts[:, 0:128] = pswap
    consts[:, 128:256] = np.eye(128, dtype=np.float32)
    consts[:, 384:448] = 1.0
    # selector matrices: bc[m, s] = recip[32*(2*jo + m//64), s]
    for jo in range(2):
        sel = np.zeros((128, 128), dtype=np.float32)
        for m in range(128):
            sel[32 * (2 * jo + m // 64), m] = 1.0
        consts[:, 512 + 128 * jo : 640 + 128 * jo] = sel
    consts = consts.astype(BF_NP)

    band = None
    if band_mode:
        # band[tp, c] = 1.0 iff (c - 384) >= tp ; slice at 384 - (t0 - s0)
        cc = np.arange(896)[None, :] - 384
        tp = np.arange(128)[:, None]
        band = (cc >= tp).astype(BF_NP)

    in_maps = []
    for c in range(N_CORES):
        q_rows = Wqkv[c * G * HD : (c + 1) * G * HD]  # [256, H]
        k_rows = Wqkv[NH * HD + c * HD : NH * HD + (c + 1) * HD]  # [64, H]
        v_rows = Wqkv[(NH + NKV) * HD + c * HD : (NH + NKV) * HD + (c + 1) * HD]
        w_loc = np.concatenate([q_rows, k_rows, v_rows], axis=0)  # [384, H]
        # wqkvT[p, k*JL + j] = w_loc[j, k*128 + p]
        wqkvT = np.ascontiguousarray(
            w_loc.T.reshape(NTILES_H, 128, JL)
            .transpose(1, 0, 2)
            .reshape(128, NTILES_H * JL)
            .astype(BF_NP)
        )
        # woT[p, jo, o] = Wo[o, c*YL + jo*128 + p]
        woT = np.ascontiguousarray(
            Wo[:, c * YL : (c + 1) * YL]
            .T.reshape(2, 128, H)
            .transpose(1, 0, 2)
            .astype(BF_NP)
        )
        m = {
            "xT": xT,
            "wqkvT": wqkvT,
            "woT": woT,
            "ctab": ctab,
            "stab": stab,
            "consts": consts,
        }
        if band is not None:
            m["band"] = band
        if mask_tiles is not None:
            m["gmask"] = mask_tiles.astype(BF_NP)
        in_maps.append(m)

    n_gen = 0 if mask_tiles is None else mask_tiles.shape[0]
    return in_maps, sched, n_gen, band_mode


def run(x, freqs_cis, mask, Wqkv, Wo, trace=False, trace_cores=None):
    from concourse.bass_utils import run_bass_kernel_spmd

    in_maps, sched, n_gen, band_mode = host_prep(x, freqs_cis, mask, Wqkv, Wo)
    nc = build_nc(sched, n_gen, band_mode)
    res = run_bass_kernel_spmd(
        nc,
        in_maps,
        list(range(N_CORES)),
        trace=trace,
        trace_cores=trace_cores,
    )
    acc = np.zeros((H, S), dtype=np.float32)
    for c in range(N_CORES):
        acc += res.results[c]["out_t"].astype(np.float32)
    out = acc.T.astype(np.float32).reshape(1, S, H)
    return out, res


_NC_CACHE = {}


def kernel(x, freqs_cis, mask, Wqkv, Wo):
    from concourse.bass_utils import run_bass_kernel_spmd

    in_maps, sched, n_gen, band_mode = host_prep(x, freqs_cis, mask, Wqkv, Wo)
    key = (
        tuple(
            tuple(e if m is None else (e, m[0], m[1]) for e, m in es)
            for es in sched
        ),
        n_gen,
        band_mode,
    )
    if key not in _NC_CACHE:
        _NC_CACHE[key] = build_nc(sched, n_gen, band_mode)
    # transient NRT_EXEC_UNIT_UNRECOVERABLE from a previously wedged
    # device clears on retry (sometimes needs two)
    for attempt in range(3):
        try:
            res = run_bass_kernel_spmd(
                _NC_CACHE[key], in_maps, list(range(N_CORES))
            )
            break
        except Exception:
            if attempt == 2:
                raise
            import time

            time.sleep(5)
    acc = np.zeros((H, S), dtype=np.float32)
    for c in range(N_CORES):
        acc += res.results[c]["out_t"].astype(np.float32)
    return acc.T.astype(np.float32).reshape(1, S, H)


# revision 46
# speedup vs baseline: 1.0174x; 1.0174x over previous
"""Trainium2 Bass kernel for nn_Attention_7911329759504 (GQA attention,
B=1, S=2048, H=2048, 32 query heads / 8 KV heads, head_dim 64, RoPE,
causal mask, fp32 in/out).

Strategy: tensor-parallel across 8 NeuronCores by KV head -- each core owns
one KV head and its 4 query heads (shards Wqkv rows / Wo columns by head),
computes a full partial output, and the host sums the 8 partials (the
"all-reduce after wo" done on the host since each core's output is a pure
summand).

All matmul operands are bf16 (PSUM accumulation fp32): enables the PE
fast-weight-load path (fp32r serialized every LDWEIGHTS ~184ns against its
matmul), halves HBM traffic, and doubles DVE throughput on elementwise ops.

PSUM layout (8 banks of [128 x 512 fp32]):
  m0, m1       -- qkv accumulators (j0/j1 pass, then j2 pass reusing m0),
                  rope swap, v-transpose, norm broadcast, wo outputs
  sc x2 buffers -- score tiles [128, 2, SCH], double-buffered so entry
                  e+1's score matmuls don't stall on exp(e) reading sct
  pvA, pvB     -- attention pv accumulators (one per head of the jo pair)

Self-contained: hardcodes all shapes; only imports concourse from the
system install. `kernel(**inputs)` takes the full unsharded inputs and
returns the full [1, S, H] float32 output.
"""

import sys

sys.path.insert(0, "/opt/trn_rl_repo")

import numpy as np
import ml_dtypes

import concourse.bass as bass
import concourse.mybir as mybir
import concourse.tile as tile

F32 = mybir.dt.float32
BF16 = mybir.dt.bfloat16
AF = mybir.ActivationFunctionType
ALU = mybir.AluOpType
BF_NP = ml_dtypes.bfloat16

S = 2048
H = 2048
NH, NKV, HD = 32, 8, 64
G = NH // NKV            # query heads per kv head = 4
JL = G * HD + 2 * HD     # local qkv rows per core = 384
YL = G * HD              # local y rows per core = 256
SCH = 512                # s-chunk (psum bank width in fp32)
NCH = S // SCH           # 4 s-chunks
NKT = S // 128           # 16 t-tiles
NTILES_H = H // 128      # 16 contraction tiles for the projections
N_CORES = 8

MAX_RESIDENT_MASKS = 5


def make_schedule(mask_np):
    """Per (s-chunk, t-tile) status from the actual [S, S] bool mask.

    Returns (sched, mask_tiles, band_mode):
      sched[chunk] = list of (ti, mask_spec or None); skipped tiles omitted.
      mask_tiles: None (band mode / no partials) or [n, 128, SCH] f32 array.
      band_mode: True when mask is exactly tril (use the shared band const).
    """
    tril = np.tril(np.ones((S, S), dtype=bool))
    band_mode = np.array_equal(mask_np, tril)
    sched = []
    tiles = []
    for c in range(NCH):
        s0 = c * SCH
        entries = []
        for ti in range(NKT):
            t0 = ti * 128
            blk = mask_np[s0 : s0 + SCH, t0 : t0 + 128]  # [s, t]
            if not blk.any():
                continue
            if blk.all():
                entries.append((ti, None))
            elif band_mode:
                # partial tile of tril: band slice at offset 384 - (t0 - s0)
                entries.append((ti, ("band", 384 - (t0 - s0))))
            else:
                tiles.append(blk.T.astype(np.float32))  # [t(128), s(SCH)]
                entries.append((ti, ("gen", len(tiles) - 1)))
        sched.append(entries)
    mask_tiles = np.stack(tiles) if tiles else None
    return sched, mask_tiles, band_mode


def build_nc(sched, n_gen_masks, band_mode):
    nc = bass.Bass(target_bir_lowering=False)

    # p-major host layouts: every DMA descriptor row is 4KB+ contiguous.
    # The DMA subsystem is packet-rate-bound (~150ns per descriptor row),
    # so [H, S]-style layouts (1KB rows) kept the queues busy end-to-end.
    xT = nc.declare_dram_parameter("xT", [128, H // 128, S], BF16, isOutput=False)
    wqkvT = nc.declare_dram_parameter(
        "wqkvT", [128, (H // 128) * JL], BF16, isOutput=False
    )
    woT = nc.declare_dram_parameter("woT", [128, 2, H], BF16, isOutput=False)
    ctab = nc.declare_dram_parameter("ctab", [128, S], BF16, isOutput=False)
    stab = nc.declare_dram_parameter("stab", [128, S], BF16, isOutput=False)
    consts = nc.declare_dram_parameter("consts", [128, 768], BF16, isOutput=False)
    # consts columns: [0:128] pswap, [128:256] identity, [384:448] ones block,
    # [512:640] sel0, [640:768] sel1 (denominator row-broadcast selectors)
    band = None
    if band_mode:
        band = nc.declare_dram_parameter("band", [128, 896], BF16, isOutput=False)
    gmask = None
    if n_gen_masks:
        gmask = nc.declare_dram_parameter(
            "gmask", [n_gen_masks, 128, SCH], BF16, isOutput=False
        )
    out_t = nc.declare_dram_parameter("out_t", [H, S], BF16, isOutput=True)

    resident_masks = bool(n_gen_masks) and n_gen_masks <= MAX_RESIDENT_MASKS
    NTILES = H // 128

    with tile.TileContext(nc) as tc:
        with (
            tc.tile_pool(name="const", bufs=1) as cpool,
            tc.tile_pool(name="psb", bufs=4) as p_pool,
            tc.tile_pool(name="tmp", bufs=2) as tmp_pool,
            tc.tile_pool(name="osb", bufs=6) as o_pool,
        ):
            # ---- persistent SBUF tensors ----
            x_sb = cpool.tile([128, NTILES, S], BF16, tag="x")
            wq_sb = cpool.tile([128, NTILES, JL], BF16, tag="wq")
            wo_sb = cpool.tile([128, 2, H], BF16, tag="wo")
            c_sb = cpool.tile([128, S], BF16, tag="ctab")
            s_sb = cpool.tile([128, S], BF16, tag="stab")
            k_sb = cpool.tile([128, 768], BF16, tag="consts")
            qkv_sb = cpool.tile([128, 3, S], BF16, tag="qkv")
            # zero-padded roped-k copies: _lo has k in rows 0:64 (even heads),
            # _hi in rows 64:128 (odd heads); opposite halves zero so score
            # matmuls run with full K=128 geometry. (2x-row-tiled K=64 score
            # pairs were tried and REGRESSED ~25%: the mode switches against
            # the 128-mode pv/filler matmuls drain the PE pipeline.)
            kdup_lo = cpool.tile([128, S], BF16, tag="kdlo")
            kdup_hi = cpool.tile([128, S], BF16, tag="kdhi")
            # v tiles padded to 128 stationary columns so FWL stays enabled
            # (cols 0:64 = v dims, col 64 = ones for the denominator row,
            # cols 65:128 zeroed once at startup)
            v_sb = cpool.tile([128, NKT, 128], BF16, tag="vt")
            y_sb = cpool.tile([128, 2, S], BF16, tag="yt")
            den_sb = cpool.tile([128, S], F32, tag="den")
            rec_sb = cpool.tile([128, S], BF16, tag="rec")
            band_sb = None
            if band_mode:
                band_sb = cpool.tile([128, 896], BF16, tag="band")
            gm_sb = None
            if resident_masks:
                gm_sb = cpool.tile([128, n_gen_masks, SCH], BF16, tag="gm")

            pswap = k_sb[:, 0:128]
            ident = k_sb[:, 128:256]

            def emit_wq_group(g):
                nc.sync.dma_start(
                    out=wq_sb[:, 4 * g : 4 * g + 4, :],
                    in_=wqkvT[:, g * 4 * JL : (g + 1) * 4 * JL],
                )

            with (
                tc.tile_pool(name="mainps", bufs=1, space="PSUM") as mps,
                tc.tile_pool(name="scps", bufs=2, space="PSUM") as sc_psum,
                tc.tile_pool(name="pvps", bufs=1, space="PSUM") as pv_psum,
            ):

                def emit_qkv_stepA(ch, k, ps01):
                    cs = slice(ch * SCH, (ch + 1) * SCH)
                    if ch == 0 and k % 4 == 0:
                        emit_wq_group(k // 4)
                    nc.sync.dma_start(out=x_sb[:, k, cs], in_=xT[:, k, cs])
                    for j in range(2):
                        nc.tensor.matmul(
                            ps01[j],
                            wq_sb[:, k, j * 128 : (j + 1) * 128],
                            x_sb[:, k, cs],
                            start=(k == 0),
                            stop=(k == NTILES - 1),
                        )

                def emit_qkv_stepB(ch, k, ps2):
                    cs = slice(ch * SCH, (ch + 1) * SCH)
                    nc.tensor.matmul(
                        ps2[0],
                        wq_sb[:, k, 256:384],
                        x_sb[:, k, cs],
                        start=(k == 0),
                        stop=(k == NTILES - 1),
                    )

                def emit_qkv_copyback(ch, j, ps):
                    cs = slice(ch * SCH, (ch + 1) * SCH)
                    nc.vector.tensor_copy(qkv_sb[:, j, cs], ps)

                def make_qkv_filler(ch):
                    ps0 = mps.tile([128, SCH], F32, tag="m0", name="qkvps0")
                    ps1 = mps.tile([128, SCH], F32, tag="m1", name="qkvps1")
                    items = [
                        (lambda k=k: emit_qkv_stepA(ch, k, (ps0, ps1)))
                        for k in range(NTILES)
                    ]
                    items.append(lambda: emit_qkv_copyback(ch, 0, ps0))
                    items.append(lambda: emit_qkv_copyback(ch, 1, ps1))
                    ps2 = []

                    def start_b():
                        ps2.append(mps.tile([128, SCH], F32, tag="m0", name="qkvps2"))

                    items.append(start_b)
                    items += [
                        (lambda k=k: emit_qkv_stepB(ch, k, ps2))
                        for k in range(NTILES)
                    ]
                    items.append(lambda: emit_qkv_copyback(ch, 2, ps2[0]))
                    return items

                def emit_wo_step(ch, ot, slots):
                    cs = slice(ch * SCH, (ch + 1) * SCH)
                    os_ = slice(ot * 128, (ot + 1) * 128)
                    wp = slots[ot % len(slots)]()
                    for jo in range(2):
                        nc.tensor.matmul(
                            wp[:],
                            wo_sb[:, jo, os_],
                            y_sb[:, jo, cs],
                            start=(jo == 0),
                            stop=(jo == 1),
                        )
                    ob = o_pool.tile([128, SCH], BF16, tag="ob", name="ob")
                    if ot % 2 == 0:
                        nc.scalar.copy(ob[:], wp[:])
                    else:
                        nc.vector.tensor_copy(ob[:], wp[:])
                    # tail stores split across both hwdge queues (ACT is idle
                    # by then); earlier chunks stay on sync so DMA issues
                    # never delay the exp stream on the scalar engine
                    if ch == NCH - 1 and ot % 2 == 1:
                        nc.scalar.dma_start(out=out_t[os_, cs], in_=ob[:])
                    else:
                        nc.sync.dma_start(out=out_t[os_, cs], in_=ob[:])

                def wo_slots_m01():
                    return [
                        lambda: mps.tile([128, SCH], F32, tag="m0", name="wops"),
                        lambda: mps.tile([128, SCH], F32, tag="m1", name="wops"),
                    ]

                def make_wo_filler(ch, slots=None):
                    if slots is None:
                        slots = wo_slots_m01()
                    return [
                        (lambda ot=ot: emit_wo_step(ch, ot, slots))
                        for ot in range(H // 128)
                    ]

                def emit_rope_jo(ch, jo):
                    cs = slice(ch * SCH, (ch + 1) * SCH)
                    pcount = 128 if jo < 2 else 64
                    swt = mps.tile([128, SCH], F32, tag="m1", name="swps")
                    # full 128-col pswap keeps NumWeights==128 (FWL); for
                    # jo=2 rows 64:128 of swt are swapped-v garbage, unread
                    nc.tensor.matmul(
                        swt[:],
                        pswap,
                        qkv_sb[:, jo, cs],
                        start=True,
                        stop=True,
                    )
                    t0 = tmp_pool.tile([128, SCH], BF16, tag="ropet0")
                    nc.vector.tensor_mul(
                        t0[:pcount], qkv_sb[:pcount, jo, cs], c_sb[:pcount, cs]
                    )
                    t1 = tmp_pool.tile([128, SCH], BF16, tag="ropet1")
                    nc.vector.tensor_mul(
                        t1[:pcount], swt[:pcount], s_sb[:pcount, cs]
                    )
                    nc.vector.tensor_add(
                        qkv_sb[:pcount, jo, cs], t0[:pcount], t1[:pcount]
                    )

                def emit_rope_kv(ch):
                    cs = slice(ch * SCH, (ch + 1) * SCH)
                    # roped k into the zero-padded lo/hi copies
                    nc.vector.tensor_copy(kdup_lo[0:64, cs], qkv_sb[0:64, 2, cs])
                    nc.vector.tensor_copy(kdup_hi[64:128, cs], qkv_sb[0:64, 2, cs])
                    # v transpose for this chunk's t-tiles
                    for kt in range(4 * ch, 4 * ch + 4):
                        tp = mps.tile([128, 64], BF16, tag="m0", name="vtps")
                        nc.tensor.transpose(
                            tp,
                            qkv_sb[64:128, 2, kt * 128 : (kt + 1) * 128],
                            ident[64:128, 64:128],
                        )
                        nc.vector.tensor_copy(v_sb[:, kt, 0:64], tp)

                def emit_rope(ch):
                    for jo in range(3):
                        emit_rope_jo(ch, jo)
                    emit_rope_kv(ch)

                def make_rope_filler(ch):
                    items = [
                        (lambda jo=jo: emit_rope_jo(ch, jo)) for jo in range(3)
                    ]
                    items.append(lambda: emit_rope_kv(ch))
                    return items

                def emit_const_dmas():
                    # issued from the (idle at startup) ACT queue so they
                    # don't delay the first xt/wq loads on the sync queue
                    nc.scalar.dma_start(out=c_sb[:], in_=ctab[:])
                    nc.scalar.dma_start(out=s_sb[:], in_=stab[:])
                    nc.scalar.dma_start(out=k_sb[:], in_=consts[:])
                    if band_mode:
                        nc.scalar.dma_start(out=band_sb[:], in_=band[:])
                    if resident_masks:
                        nc.scalar.dma_start(
                            out=gm_sb[:], in_=gmask.rearrange("n p f -> p n f")
                        )
                    nc.scalar.dma_start(out=wo_sb[:], in_=woT[:])

                def emit_const_init():
                    # den_sb := 1.0, rec_sb := 0 (garbage rows must stay
                    # finite: sel matmuls read the full rec partition range)
                    nc.vector.tensor_scalar(
                        den_sb[:], c_sb[:], 0.0, 1.0, ALU.mult, ALU.add
                    )
                    nc.vector.tensor_scalar(
                        rec_sb[:], c_sb[:], 0.0, 0.0, ALU.mult, ALU.mult
                    )
                    # ones column of v_hat; zero pad cols; zero halves of the
                    # k copies
                    nc.vector.tensor_copy(v_sb[:, :, 64], k_sb[:, 384 : 384 + NKT])
                    for kt in range(NKT):
                        nc.vector.tensor_scalar(
                            v_sb[:, kt, 65:128], c_sb[:, 0:63], 0.0, 0.0,
                            ALU.mult, ALU.mult,
                        )
                    nc.scalar.activation(
                        kdup_lo[64:128, :], c_sb[64:128, :], AF.Copy, scale=0.0
                    )
                    nc.scalar.activation(
                        kdup_hi[0:64, :], c_sb[0:64, :], AF.Copy, scale=0.0
                    )

                def emit_attn(ch, filler, filler_jo1=()):
                    """Attention for chunk ch; filler items are interleaved
                    into the t-loop to keep the PE fed while exp runs.
                    filler_jo1 items are only drained during the jo=1 pass
                    (they depend on jo=0's outputs)."""
                    cs = slice(ch * SCH, (ch + 1) * SCH)
                    entries = sched[ch]
                    queue = list(filler)
                    n_iters = max(2 * len(entries), 1)
                    per_iter = -(-(len(queue) + len(filler_jo1)) // n_iters)
                    state = {"idx": 0}

                    def drain_filler(n):
                        for _ in range(n):
                            if state["idx"] < len(queue):
                                queue[state["idx"]]()
                                state["idx"] += 1

                    for jo in range(2):
                        if not entries:
                            continue
                        if jo == 1:
                            queue.extend(filler_jo1)
                        pvA = pv_psum.tile([128, SCH], F32, tag="pvA", name="pvA")
                        pvB = pv_psum.tile([128, SCH], F32, tag="pvB", name="pvB")
                        pvs = (pvA, pvB)
                        pending = []

                        def flush_pending(keep):
                            while len(pending) > keep:
                                pp, pei = pending.pop(0)
                                for hp in range(2):
                                    nc.tensor.matmul(
                                        pvs[hp][:],
                                        v_sb[:, entries[pei][0], :],
                                        pp[:, hp, :],
                                        start=(pei == 0),
                                        stop=(pei == len(entries) - 1),
                                    )

                        PENDING_DEPTH = 1
                        for ei, (ti, mk) in enumerate(entries):
                            sct = sc_psum.tile(
                                [128, 2, SCH], F32, tag="sc", name="sc"
                            )
                            tc_sl = slice(ti * 128, (ti + 1) * 128)
                            nc.tensor.matmul(
                                sct[:, 0, :], kdup_lo[:, tc_sl],
                                qkv_sb[:, jo, cs], start=True, stop=True,
                            )
                            nc.tensor.matmul(
                                sct[:, 1, :], kdup_hi[:, tc_sl],
                                qkv_sb[:, jo, cs], start=True, stop=True,
                            )
                            p_big = p_pool.tile(
                                [128, 2, SCH], BF16, tag="p", name="p"
                            )
                            nc.scalar.activation(
                                p_big[:], sct[:], AF.Exp, scale=0.125
                            )
                            if mk is not None:
                                kind, arg = mk
                                for hp in range(2):
                                    if kind == "band":
                                        nc.vector.tensor_mul(
                                            p_big[:, hp, :], p_big[:, hp, :],
                                            band_sb[:, arg : arg + SCH],
                                        )
                                    elif resident_masks:
                                        nc.vector.tensor_mul(
                                            p_big[:, hp, :], p_big[:, hp, :],
                                            gm_sb[:, arg, :],
                                        )
                                    else:
                                        mt = tmp_pool.tile(
                                            [128, SCH], BF16, tag="mstream"
                                        )
                                        nc.sync.dma_start(
                                            out=mt[:], in_=gmask[arg]
                                        )
                                        nc.vector.tensor_mul(
                                            p_big[:, hp, :], p_big[:, hp, :],
                                            mt[:],
                                        )
                            pending.append((p_big, ei))
                            flush_pending(PENDING_DEPTH)
                            drain_filler(per_iter)
                        flush_pending(0)
                        # unnormalized y (cross-base for odd heads) + den rows
                        for hp in range(2):
                            h = 2 * jo + hp
                            bp = hp * 64
                            nc.vector.tensor_copy(
                                y_sb[bp : bp + 64, jo, cs], pvs[hp][0:64]
                            )
                            nc.vector.tensor_copy(
                                den_sb[32 * h : 32 * h + 1, cs], pvs[hp][64:65]
                            )
                    drain_filler(len(queue))

                def emit_norm_jo(ch, jo):
                    cs = slice(ch * SCH, (ch + 1) * SCH)
                    rs = slice(64 * jo, 64 * jo + 64)
                    # 1/x = exp(-ln(x)): DVE reciprocal on few partitions is
                    # pathologically slow; ACT ln+exp is flat-rate
                    nc.scalar.activation(den_sb[rs, cs], den_sb[rs, cs], AF.Ln)
                    nc.scalar.activation(
                        rec_sb[rs, cs], den_sb[rs, cs], AF.Exp, scale=-1.0
                    )
                    sel = k_sb[:, 512 + 128 * jo : 640 + 128 * jo]
                    bct = mps.tile([128, SCH], F32, tag="m1", name="bcps")
                    nc.tensor.matmul(
                        bct[:], sel, rec_sb[:, cs], start=True, stop=True
                    )
                    nc.vector.tensor_mul(y_sb[:, jo, cs], y_sb[:, jo, cs], bct[:])

                def make_norm_filler(ch):
                    return [
                        lambda: emit_norm_jo(ch, 0),
                        lambda: emit_norm_jo(ch, 1),
                    ]

                # ---- prologue: qkv(0) + consts + rope(0); later qkv
                # chunks ride inside the attention loops as PE filler ----
                emit_const_dmas()
                for item in make_qkv_filler(0):
                    item()
                emit_const_init()
                emit_rope(0)

                # ---- main loop: attn(c) with later qkv and wo(c-1) woven
                # in; wo chunks spread over attn(2) and attn(3) so the out
                # stores stream early instead of piling up at the end ----
                for c in range(NCH):
                    filler = []
                    filler_jo1 = ()
                    if 1 <= c <= 2:
                        filler += make_norm_filler(c - 1)
                    if c + 1 < NCH:
                        filler += make_qkv_filler(c + 1)
                        filler += make_rope_filler(c + 1)
                        if c == 2:
                            filler += make_wo_filler(0)
                    else:
                        filler += make_norm_filler(c - 1)
                        filler += make_wo_filler(1)
                        filler += make_wo_filler(2)
                        filler_jo1 = [lambda: emit_norm_jo(NCH - 1, 0)]
                    emit_attn(c, filler, filler_jo1)
                    if c == NCH - 1:
                        emit_norm_jo(c, 1)

                # ---- tail: wo(3) across m0/m1 + both sc buffers' halves ----
                tail_sct1 = sc_psum.tile([128, 2, SCH], F32, tag="sc", name="sc")
                tail_sct2 = sc_psum.tile([128, 2, SCH], F32, tag="sc", name="sc")
                tail_slots = wo_slots_m01() + [
                    (lambda: tail_sct1[:, 0, :]),
                    (lambda: tail_sct1[:, 1, :]),
                    (lambda: tail_sct2[:, 0, :]),
                    (lambda: tail_sct2[:, 1, :]),
                ]
                for item in make_wo_filler(NCH - 1, slots=tail_slots):
                    item()

    fixup_multi_waits(nc)
    return nc


def fixup_multi_waits(nc):
    """walrus CoreV2/V3 codegen rejects instructions carrying more than one
    sync wait. Split extra waits onto same-engine NoOps inserted before."""
    n_split = 0
    for fn in nc.m.functions:
        for bb in fn.blocks:
            new_insts = []
            for inst in bb.instructions:
                si = inst.sync_info
                if si is not None and si.on_wait and len(si.on_wait) > 1:
                    waits = list(si.on_wait)
                    for w in waits[:-1]:
                        n_split += 1
                        nop = mybir.InstNoOp(
                            name=f"I-waitsplit-{n_split}",
                            engine=inst.engine,
                            ins=[],
                            outs=[],
                            sync_info=mybir.SyncInfo(on_wait=[w], on_update=[]),
                        )
                        new_insts.append(nop)
                    si.on_wait = [waits[-1]]
                new_insts.append(inst)
            bb.instructions[:] = new_insts
    return n_split


def host_prep(x, freqs_cis, mask, Wqkv, Wo):
    """Build per-core input maps + the shared schedule."""
    x = np.asarray(x, dtype=np.float32)
    freqs_cis = np.asarray(freqs_cis, dtype=np.float32)
    mask_np = np.asarray(mask).reshape(S, S).astype(bool)
    Wqkv = np.asarray(Wqkv, dtype=np.float32)
    Wo = np.asarray(Wo, dtype=np.float32)

    sched, mask_tiles, band_mode = make_schedule(mask_np)

    # p-major: xT[p, k, s] = x[s, k*128 + p]; per-partition rows contiguous
    xT = np.ascontiguousarray(
        x.reshape(S, H).T.reshape(NTILES_H, 128, S).transpose(1, 0, 2).astype(BF_NP)
    )

    cos_t = np.ascontiguousarray(freqs_cis[:, :, 0].T)  # [32, S]
    sin_t = np.ascontiguousarray(freqs_cis[:, :, 1].T)
    c64 = np.repeat(cos_t, 2, axis=0)  # [64, S]
    s64 = np.repeat(sin_t, 2, axis=0)
    ctab = np.tile(c64, (2, 1)).astype(BF_NP)  # [128, S]
    stab = np.tile(s64, (2, 1)).astype(BF_NP)

    # pswap: out[m] = -in[m+1] (m even), +in[m-1] (m odd); lhsT[k, m]
    pswap = np.zeros((128, 128), dtype=np.float32)
    for i in range(64):
        pswap[2 * i + 1, 2 * i] = -1.0
        pswap[2 * i, 2 * i + 1] = 1.0
    consts = np.zeros((128, 768), dtype=np.float32)
    consts[:, 0:128] = pswap
    consts[:, 128:256] = np.eye(128, dtype=np.float32)
    consts[:, 384:448] = 1.0
    # selector matrices: bc[m, s] = recip[32*(2*jo + m//64), s]
    for jo in range(2):
        sel = np.zeros((128, 128), dtype=np.float32)
        for m in range(128):
            sel[32 * (2 * jo + m // 64), m] = 1.0
        consts[:, 512 + 128 * jo : 640 + 128 * jo] = sel
    consts = consts.astype(BF_NP)

    band = None
    if band_mode:
        # band[tp, c] = 1.0 iff (c - 384) >= tp ; slice at 384 - (t0 - s0)
        cc = np.arange(896)[None, :] - 384
        tp = np.arange(128)[:, None]
        band = (cc >= tp).astype(BF_NP)

    in_maps = []
    for c in range(N_CORES):
        q_rows = Wqkv[c * G * HD : (c + 1) * G * HD]  # [256, H]
        k_rows = Wqkv[NH * HD + c * HD : NH * HD + (c + 1) * HD]  # [64, H]
        v_rows = Wqkv[(NH + NKV) * HD + c * HD : (NH + NKV) * HD + (c + 1) * HD]
        w_loc = np.concatenate([q_rows, k_rows, v_rows], axis=0)  # [384, H]
        # wqkvT[p, k*JL + j] = w_loc[j, k*128 + p]
        wqkvT = np.ascontiguousarray(
            w_loc.T.reshape(NTILES_H, 128, JL)
            .transpose(1, 0, 2)
            .reshape(128, NTILES_H * JL)
            .astype(BF_NP)
        )
        # woT[p, jo, o] = Wo[o, c*YL + jo*128 + p]
        woT = np.ascontiguousarray(
            Wo[:, c * YL : (c + 1) * YL]
            .T.reshape(2, 128, H)
            .transpose(1, 0, 2)
            .astype(BF_NP)
        )
        m = {
            "xT": xT,
            "wqkvT": wqkvT,
            "woT": woT,
            "ctab": ctab,
            "stab": stab,
            "consts": consts,
        }
        if band is not None:
            m["band"] = band
        if mask_tiles is not None:
            m["gmask"] = mask_tiles.astype(BF_NP)
        in_maps.append(m)

    n_gen = 0 if mask_tiles is None else mask_tiles.shape[0]
    return in_maps, sched, n_gen, band_mode


def run(x, freqs_cis, mask, Wqkv, Wo, trace=False, trace_cores=None):
    from concourse.bass_utils import run_bass_kernel_spmd

    in_maps, sched, n_gen, band_mode = host_prep(x, freqs_cis, mask, Wqkv, Wo)
    nc = build_nc(sched, n_gen, band_mode)
    res = run_bass_kernel_spmd(
        nc,
        in_maps,
        list(range(N_CORES)),
        trace=trace,
        trace_cores=trace_cores,
    )
    acc = np.zeros((H, S), dtype=np.float32)
    for c in range(N_CORES):
        acc += res.results[c]["out_t"].astype(np.float32)
    out = acc.T.astype(np.float32).reshape(1, S, H)
    return out, res


_NC_CACHE = {}


def kernel(x, freqs_cis, mask, Wqkv, Wo):
    from concourse.bass_utils import run_bass_kernel_spmd

    in_maps, sched, n_gen, band_mode = host_prep(x, freqs_cis, mask, Wqkv, Wo)
    key = (
        tuple(
            tuple(e if m is None else (e, m[0], m[1]) for e, m in es)
            for es in sched
        ),
        n_gen,
        band_mode,
    )
    if key not in _NC_CACHE:
        _NC_CACHE[key] = build_nc(sched, n_gen, band_mode)
    # transient NRT_EXEC_UNIT_UNRECOVERABLE from a previously wedged
    # device clears on retry (sometimes needs two)
    for attempt in range(3):
        try:
            res = run_bass_kernel_spmd(
                _NC_CACHE[key], in_maps, list(range(N_CORES))
            )
            break
        except Exception:
            if attempt == 2:
                raise
            import time

            time.sleep(5)
    acc = np.zeros((H, S), dtype=np.float32)
    for c in range(N_CORES):
        acc += res.results[c]["out_t"].astype(np.float32)
    return acc.T.astype(np.float32).reshape(1, S, H)


# revision 47
# speedup vs baseline: 1.0356x; 1.0179x over previous
"""Trainium2 Bass kernel for nn_Attention_7911329759504 (GQA attention,
B=1, S=2048, H=2048, 32 query heads / 8 KV heads, head_dim 64, RoPE,
causal mask, fp32 in/out).

Strategy: tensor-parallel across 8 NeuronCores by KV head -- each core owns
one KV head and its 4 query heads (shards Wqkv rows / Wo columns by head),
computes a full partial output, and the host sums the 8 partials (the
"all-reduce after wo" done on the host since each core's output is a pure
summand).

All matmul operands are bf16 (PSUM accumulation fp32): enables the PE
fast-weight-load path (fp32r serialized every LDWEIGHTS ~184ns against its
matmul), halves HBM traffic, and doubles DVE throughput on elementwise ops.

PSUM layout (8 banks of [128 x 512 fp32]):
  m0, m1       -- qkv accumulators (j0/j1 pass, then j2 pass reusing m0),
                  rope swap, v-transpose, norm broadcast, wo outputs
  sc x2 buffers -- score tiles [128, 2, SCH], double-buffered so entry
                  e+1's score matmuls don't stall on exp(e) reading sct
  pvA, pvB     -- attention pv accumulators (one per head of the jo pair)

Self-contained: hardcodes all shapes; only imports concourse from the
system install. `kernel(**inputs)` takes the full unsharded inputs and
returns the full [1, S, H] float32 output.
"""

import sys

sys.path.insert(0, "/opt/trn_rl_repo")

import numpy as np
import ml_dtypes

import concourse.bass as bass
import concourse.mybir as mybir
import concourse.tile as tile

F32 = mybir.dt.float32
BF16 = mybir.dt.bfloat16
AF = mybir.ActivationFunctionType
ALU = mybir.AluOpType
BF_NP = ml_dtypes.bfloat16

S = 2048
H = 2048
NH, NKV, HD = 32, 8, 64
G = NH // NKV            # query heads per kv head = 4
JL = G * HD + 2 * HD     # local qkv rows per core = 384
YL = G * HD              # local y rows per core = 256
SCH = 512                # s-chunk (psum bank width in fp32)
NCH = S // SCH           # 4 s-chunks
NKT = S // 128           # 16 t-tiles
NTILES_H = H // 128      # 16 contraction tiles for the projections
N_CORES = 8

MAX_RESIDENT_MASKS = 5


def make_schedule(mask_np):
    """Per (s-chunk, t-tile) status from the actual [S, S] bool mask.

    Returns (sched, mask_tiles, band_mode):
      sched[chunk] = list of (ti, mask_spec or None); skipped tiles omitted.
      mask_tiles: None (band mode / no partials) or [n, 128, SCH] f32 array.
      band_mode: True when mask is exactly tril (use the shared band const).
    """
    tril = np.tril(np.ones((S, S), dtype=bool))
    band_mode = np.array_equal(mask_np, tril)
    sched = []
    tiles = []
    for c in range(NCH):
        s0 = c * SCH
        entries = []
        for ti in range(NKT):
            t0 = ti * 128
            blk = mask_np[s0 : s0 + SCH, t0 : t0 + 128]  # [s, t]
            if not blk.any():
                continue
            if blk.all():
                entries.append((ti, None))
            elif band_mode:
                # partial tile of tril: band slice at offset 384 - (t0 - s0)
                entries.append((ti, ("band", 384 - (t0 - s0))))
            else:
                tiles.append(blk.T.astype(np.float32))  # [t(128), s(SCH)]
                entries.append((ti, ("gen", len(tiles) - 1)))
        sched.append(entries)
    mask_tiles = np.stack(tiles) if tiles else None
    return sched, mask_tiles, band_mode


def build_nc(sched, n_gen_masks, band_mode):
    nc = bass.Bass(target_bir_lowering=False)

    # p-major host layouts: every DMA descriptor row is 4KB+ contiguous.
    # The DMA subsystem is packet-rate-bound (~150ns per descriptor row),
    # so [H, S]-style layouts (1KB rows) kept the queues busy end-to-end.
    xT = nc.declare_dram_parameter("xT", [128, H // 128, S], BF16, isOutput=False)
    wqkvT = nc.declare_dram_parameter(
        "wqkvT", [128, (H // 128) * JL], BF16, isOutput=False
    )
    woT = nc.declare_dram_parameter("woT", [128, 2, H], BF16, isOutput=False)
    ctab = nc.declare_dram_parameter("ctab", [128, S], BF16, isOutput=False)
    stab = nc.declare_dram_parameter("stab", [128, S], BF16, isOutput=False)
    consts = nc.declare_dram_parameter("consts", [128, 768], BF16, isOutput=False)
    # consts columns: [0:128] pswap, [128:256] identity, [384:448] ones block,
    # [512:640] sel0, [640:768] sel1 (denominator row-broadcast selectors)
    band = None
    if band_mode:
        band = nc.declare_dram_parameter("band", [128, 896], BF16, isOutput=False)
    gmask = None
    if n_gen_masks:
        gmask = nc.declare_dram_parameter(
            "gmask", [n_gen_masks, 128, SCH], BF16, isOutput=False
        )
    out_t = nc.declare_dram_parameter("out_t", [H, S], BF16, isOutput=True)

    resident_masks = bool(n_gen_masks) and n_gen_masks <= MAX_RESIDENT_MASKS
    NTILES = H // 128

    with tile.TileContext(nc) as tc:
        with (
            tc.tile_pool(name="const", bufs=1) as cpool,
            tc.tile_pool(name="psb", bufs=4) as p_pool,
            tc.tile_pool(name="tmp", bufs=2) as tmp_pool,
            tc.tile_pool(name="osb", bufs=6) as o_pool,
        ):
            # ---- persistent SBUF tensors ----
            x_sb = cpool.tile([128, NTILES, S], BF16, tag="x")
            wq_sb = cpool.tile([128, NTILES, JL], BF16, tag="wq")
            wo_sb = cpool.tile([128, 2, H], BF16, tag="wo")
            c_sb = cpool.tile([128, S], BF16, tag="ctab")
            s_sb = cpool.tile([128, S], BF16, tag="stab")
            k_sb = cpool.tile([128, 768], BF16, tag="consts")
            qkv_sb = cpool.tile([128, 3, S], BF16, tag="qkv")
            # zero-padded roped-k copies: _lo has k in rows 0:64 (even heads),
            # _hi in rows 64:128 (odd heads); opposite halves zero so score
            # matmuls run with full K=128 geometry. (2x-row-tiled K=64 score
            # pairs were tried and REGRESSED ~25%: the mode switches against
            # the 128-mode pv/filler matmuls drain the PE pipeline.)
            kdup_lo = cpool.tile([128, S], BF16, tag="kdlo")
            kdup_hi = cpool.tile([128, S], BF16, tag="kdhi")
            # v tiles padded to 128 stationary columns so FWL stays enabled
            # (cols 0:64 = v dims, col 64 = ones for the denominator row,
            # cols 65:128 zeroed once at startup)
            v_sb = cpool.tile([128, NKT, 128], BF16, tag="vt")
            y_sb = cpool.tile([128, 2, S], BF16, tag="yt")
            den_sb = cpool.tile([128, S], F32, tag="den")
            rec_sb = cpool.tile([128, S], BF16, tag="rec")
            band_sb = None
            if band_mode:
                band_sb = cpool.tile([128, 896], BF16, tag="band")
            gm_sb = None
            if resident_masks:
                gm_sb = cpool.tile([128, n_gen_masks, SCH], BF16, tag="gm")

            pswap = k_sb[:, 0:128]
            ident = k_sb[:, 128:256]

            def emit_wq_group(g):
                nc.sync.dma_start(
                    out=wq_sb[:, 4 * g : 4 * g + 4, :],
                    in_=wqkvT[:, g * 4 * JL : (g + 1) * 4 * JL],
                )

            with (
                tc.tile_pool(name="mainps", bufs=1, space="PSUM") as mps,
                tc.tile_pool(name="scps", bufs=2, space="PSUM") as sc_psum,
                tc.tile_pool(name="pvps", bufs=1, space="PSUM") as pv_psum,
            ):

                def emit_qkv_stepA(ch, k, ps01):
                    cs = slice(ch * SCH, (ch + 1) * SCH)
                    if ch == 0 and k % 4 == 0:
                        emit_wq_group(k // 4)
                    nc.sync.dma_start(out=x_sb[:, k, cs], in_=xT[:, k, cs])
                    for j in range(2):
                        nc.tensor.matmul(
                            ps01[j],
                            wq_sb[:, k, j * 128 : (j + 1) * 128],
                            x_sb[:, k, cs],
                            start=(k == 0),
                            stop=(k == NTILES - 1),
                        )

                def emit_qkv_stepB(ch, k, ps2):
                    cs = slice(ch * SCH, (ch + 1) * SCH)
                    nc.tensor.matmul(
                        ps2[0],
                        wq_sb[:, k, 256:384],
                        x_sb[:, k, cs],
                        start=(k == 0),
                        stop=(k == NTILES - 1),
                    )

                def emit_qkv_copyback(ch, j, ps):
                    cs = slice(ch * SCH, (ch + 1) * SCH)
                    nc.vector.tensor_copy(qkv_sb[:, j, cs], ps)

                def make_qkv_filler(ch):
                    ps0 = mps.tile([128, SCH], F32, tag="m0", name="qkvps0")
                    ps1 = mps.tile([128, SCH], F32, tag="m1", name="qkvps1")
                    items = [
                        (lambda k=k: emit_qkv_stepA(ch, k, (ps0, ps1)))
                        for k in range(NTILES)
                    ]
                    items.append(lambda: emit_qkv_copyback(ch, 0, ps0))
                    items.append(lambda: emit_qkv_copyback(ch, 1, ps1))
                    ps2 = []

                    def start_b():
                        ps2.append(mps.tile([128, SCH], F32, tag="m0", name="qkvps2"))

                    items.append(start_b)
                    items += [
                        (lambda k=k: emit_qkv_stepB(ch, k, ps2))
                        for k in range(NTILES)
                    ]
                    items.append(lambda: emit_qkv_copyback(ch, 2, ps2[0]))
                    return items

                def emit_wo_step(ch, ot, slots):
                    cs = slice(ch * SCH, (ch + 1) * SCH)
                    os_ = slice(ot * 128, (ot + 1) * 128)
                    wp = slots[ot % len(slots)]()
                    for jo in range(2):
                        nc.tensor.matmul(
                            wp[:],
                            wo_sb[:, jo, os_],
                            y_sb[:, jo, cs],
                            start=(jo == 0),
                            stop=(jo == 1),
                        )
                    ob = o_pool.tile([128, SCH], BF16, tag="ob", name="ob")
                    if ot % 2 == 0:
                        nc.scalar.copy(ob[:], wp[:])
                    else:
                        nc.vector.tensor_copy(ob[:], wp[:])
                    # tail stores split across both hwdge queues (ACT is idle
                    # by then); earlier chunks stay on sync so DMA issues
                    # never delay the exp stream on the scalar engine
                    if ch == NCH - 1 and ot % 2 == 1:
                        nc.scalar.dma_start(out=out_t[os_, cs], in_=ob[:])
                    else:
                        nc.sync.dma_start(out=out_t[os_, cs], in_=ob[:])

                def wo_slots_m01():
                    return [
                        lambda: mps.tile([128, SCH], F32, tag="m0", name="wops"),
                        lambda: mps.tile([128, SCH], F32, tag="m1", name="wops"),
                    ]

                def make_wo_filler(ch, slots=None):
                    if slots is None:
                        slots = wo_slots_m01()
                    return [
                        (lambda ot=ot: emit_wo_step(ch, ot, slots))
                        for ot in range(H // 128)
                    ]

                def emit_rope_jo(ch, jo):
                    cs = slice(ch * SCH, (ch + 1) * SCH)
                    pcount = 128 if jo < 2 else 64
                    swt = mps.tile([128, SCH], F32, tag="m1", name="swps")
                    # full 128-col pswap keeps NumWeights==128 (FWL); for
                    # jo=2 rows 64:128 of swt are swapped-v garbage, unread
                    nc.tensor.matmul(
                        swt[:],
                        pswap,
                        qkv_sb[:, jo, cs],
                        start=True,
                        stop=True,
                    )
                    t0 = tmp_pool.tile([128, SCH], BF16, tag="ropet0")
                    nc.vector.tensor_mul(
                        t0[:pcount], qkv_sb[:pcount, jo, cs], c_sb[:pcount, cs]
                    )
                    t1 = tmp_pool.tile([128, SCH], BF16, tag="ropet1")
                    nc.vector.tensor_mul(
                        t1[:pcount], swt[:pcount], s_sb[:pcount, cs]
                    )
                    nc.vector.tensor_add(
                        qkv_sb[:pcount, jo, cs], t0[:pcount], t1[:pcount]
                    )

                def emit_rope_kv(ch):
                    cs = slice(ch * SCH, (ch + 1) * SCH)
                    # roped k into the zero-padded lo/hi copies
                    nc.vector.tensor_copy(kdup_lo[0:64, cs], qkv_sb[0:64, 2, cs])
                    nc.vector.tensor_copy(kdup_hi[64:128, cs], qkv_sb[0:64, 2, cs])
                    # v transpose for this chunk's t-tiles
                    for kt in range(4 * ch, 4 * ch + 4):
                        tp = mps.tile([128, 64], BF16, tag="m0", name="vtps")
                        nc.tensor.transpose(
                            tp,
                            qkv_sb[64:128, 2, kt * 128 : (kt + 1) * 128],
                            ident[64:128, 64:128],
                        )
                        nc.vector.tensor_copy(v_sb[:, kt, 0:64], tp)

                def emit_rope(ch):
                    for jo in range(3):
                        emit_rope_jo(ch, jo)
                    emit_rope_kv(ch)

                def make_rope_filler(ch):
                    items = [
                        (lambda jo=jo: emit_rope_jo(ch, jo)) for jo in range(3)
                    ]
                    items.append(lambda: emit_rope_kv(ch))
                    return items

                def emit_const_dmas():
                    # issued from the (idle at startup) ACT queue so they
                    # don't delay the first xt/wq loads on the sync queue
                    nc.scalar.dma_start(out=c_sb[:], in_=ctab[:])
                    nc.scalar.dma_start(out=s_sb[:], in_=stab[:])
                    nc.scalar.dma_start(out=k_sb[:], in_=consts[:])
                    if band_mode:
                        nc.scalar.dma_start(out=band_sb[:], in_=band[:])
                    if resident_masks:
                        nc.scalar.dma_start(
                            out=gm_sb[:], in_=gmask.rearrange("n p f -> p n f")
                        )
                    nc.scalar.dma_start(out=wo_sb[:], in_=woT[:])

                def emit_const_init():
                    # den_sb := 1.0, rec_sb := 0 (garbage rows must stay
                    # finite: sel matmuls read the full rec partition range)
                    nc.vector.tensor_scalar(
                        den_sb[:], c_sb[:], 0.0, 1.0, ALU.mult, ALU.add
                    )
                    nc.vector.tensor_scalar(
                        rec_sb[:], c_sb[:], 0.0, 0.0, ALU.mult, ALU.mult
                    )
                    # ones column of v_hat; zero pad cols; zero halves of the
                    # k copies
                    nc.vector.tensor_copy(v_sb[:, :, 64], k_sb[:, 384 : 384 + NKT])
                    for kt in range(NKT):
                        nc.vector.tensor_scalar(
                            v_sb[:, kt, 65:128], c_sb[:, 0:63], 0.0, 0.0,
                            ALU.mult, ALU.mult,
                        )
                    nc.scalar.activation(
                        kdup_lo[64:128, :], c_sb[64:128, :], AF.Copy, scale=0.0
                    )
                    nc.scalar.activation(
                        kdup_hi[0:64, :], c_sb[0:64, :], AF.Copy, scale=0.0
                    )

                def emit_attn(ch, filler, filler_jo1=()):
                    """Attention for chunk ch; filler items are interleaved
                    into the t-loop to keep the PE fed while exp runs.
                    filler_jo1 items are only drained during the jo=1 pass
                    (they depend on jo=0's outputs)."""
                    cs = slice(ch * SCH, (ch + 1) * SCH)
                    entries = sched[ch]
                    queue = list(filler)
                    n_iters = max(2 * len(entries), 1)
                    per_iter = -(-(len(queue) + len(filler_jo1)) // n_iters)
                    state = {"idx": 0}

                    def drain_filler(n):
                        for _ in range(n):
                            if state["idx"] < len(queue):
                                queue[state["idx"]]()
                                state["idx"] += 1

                    for jo in range(2):
                        if not entries:
                            continue
                        if jo == 1:
                            queue.extend(filler_jo1)
                        pvA = pv_psum.tile([128, SCH], F32, tag="pvA", name="pvA")
                        pvB = pv_psum.tile([128, SCH], F32, tag="pvB", name="pvB")
                        pvs = (pvA, pvB)
                        pending = []

                        def flush_pending(keep):
                            while len(pending) > keep:
                                pp, pei = pending.pop(0)
                                for hp in range(2):
                                    nc.tensor.matmul(
                                        pvs[hp][:],
                                        v_sb[:, entries[pei][0], :],
                                        pp[:, hp, :],
                                        start=(pei == 0),
                                        stop=(pei == len(entries) - 1),
                                    )

                        PENDING_DEPTH = 2
                        for ei, (ti, mk) in enumerate(entries):
                            sct = sc_psum.tile(
                                [128, 2, SCH], F32, tag="sc", name="sc"
                            )
                            tc_sl = slice(ti * 128, (ti + 1) * 128)
                            nc.tensor.matmul(
                                sct[:, 0, :], kdup_lo[:, tc_sl],
                                qkv_sb[:, jo, cs], start=True, stop=True,
                            )
                            nc.tensor.matmul(
                                sct[:, 1, :], kdup_hi[:, tc_sl],
                                qkv_sb[:, jo, cs], start=True, stop=True,
                            )
                            p_big = p_pool.tile(
                                [128, 2, SCH], BF16, tag="p", name="p"
                            )
                            nc.scalar.activation(
                                p_big[:], sct[:], AF.Exp, scale=0.125
                            )
                            if mk is not None:
                                kind, arg = mk
                                for hp in range(2):
                                    if kind == "band":
                                        nc.vector.tensor_mul(
                                            p_big[:, hp, :], p_big[:, hp, :],
                                            band_sb[:, arg : arg + SCH],
                                        )
                                    elif resident_masks:
                                        nc.vector.tensor_mul(
                                            p_big[:, hp, :], p_big[:, hp, :],
                                            gm_sb[:, arg, :],
                                        )
                                    else:
                                        mt = tmp_pool.tile(
                                            [128, SCH], BF16, tag="mstream"
                                        )
                                        nc.sync.dma_start(
                                            out=mt[:], in_=gmask[arg]
                                        )
                                        nc.vector.tensor_mul(
                                            p_big[:, hp, :], p_big[:, hp, :],
                                            mt[:],
                                        )
                            pending.append((p_big, ei))
                            flush_pending(PENDING_DEPTH)
                            drain_filler(per_iter)
                        flush_pending(0)
                        # unnormalized y (cross-base for odd heads) + den rows
                        for hp in range(2):
                            h = 2 * jo + hp
                            bp = hp * 64
                            nc.vector.tensor_copy(
                                y_sb[bp : bp + 64, jo, cs], pvs[hp][0:64]
                            )
                            nc.vector.tensor_copy(
                                den_sb[32 * h : 32 * h + 1, cs], pvs[hp][64:65]
                            )
                    drain_filler(len(queue))

                def emit_norm_jo(ch, jo):
                    cs = slice(ch * SCH, (ch + 1) * SCH)
                    rs = slice(64 * jo, 64 * jo + 64)
                    # 1/x = exp(-ln(x)): DVE reciprocal on few partitions is
                    # pathologically slow; ACT ln+exp is flat-rate
                    nc.scalar.activation(den_sb[rs, cs], den_sb[rs, cs], AF.Ln)
                    nc.scalar.activation(
                        rec_sb[rs, cs], den_sb[rs, cs], AF.Exp, scale=-1.0
                    )
                    sel = k_sb[:, 512 + 128 * jo : 640 + 128 * jo]
                    bct = mps.tile([128, SCH], F32, tag="m1", name="bcps")
                    nc.tensor.matmul(
                        bct[:], sel, rec_sb[:, cs], start=True, stop=True
                    )
                    nc.vector.tensor_mul(y_sb[:, jo, cs], y_sb[:, jo, cs], bct[:])

                def make_norm_filler(ch):
                    return [
                        lambda: emit_norm_jo(ch, 0),
                        lambda: emit_norm_jo(ch, 1),
                    ]

                # ---- prologue: qkv(0) + consts + rope(0); later qkv
                # chunks ride inside the attention loops as PE filler ----
                emit_const_dmas()
                for item in make_qkv_filler(0):
                    item()
                emit_const_init()
                emit_rope(0)

                # ---- main loop: attn(c) with later qkv and wo(c-1) woven
                # in; wo chunks spread over attn(2) and attn(3) so the out
                # stores stream early instead of piling up at the end ----
                for c in range(NCH):
                    filler = []
                    filler_jo1 = ()
                    if 1 <= c <= 2:
                        filler += make_norm_filler(c - 1)
                    if c + 1 < NCH:
                        filler += make_qkv_filler(c + 1)
                        filler += make_rope_filler(c + 1)
                        if c == 2:
                            filler += make_wo_filler(0)
                    else:
                        filler += make_norm_filler(c - 1)
                        filler += make_wo_filler(1)
                        filler += make_wo_filler(2)
                        filler_jo1 = [lambda: emit_norm_jo(NCH - 1, 0)]
                    emit_attn(c, filler, filler_jo1)
                    if c == NCH - 1:
                        emit_norm_jo(c, 1)

                # ---- tail: wo(3) across m0/m1 + both sc buffers' halves ----
                tail_sct1 = sc_psum.tile([128, 2, SCH], F32, tag="sc", name="sc")
                tail_sct2 = sc_psum.tile([128, 2, SCH], F32, tag="sc", name="sc")
                tail_slots = wo_slots_m01() + [
                    (lambda: tail_sct1[:, 0, :]),
                    (lambda: tail_sct1[:, 1, :]),
                    (lambda: tail_sct2[:, 0, :]),
                    (lambda: tail_sct2[:, 1, :]),
                ]
                for item in make_wo_filler(NCH - 1, slots=tail_slots):
                    item()

    fixup_multi_waits(nc)
    return nc


def fixup_multi_waits(nc):
    """walrus CoreV2/V3 codegen rejects instructions carrying more than one
    sync wait. Split extra waits onto same-engine NoOps inserted before."""
    n_split = 0
    for fn in nc.m.functions:
        for bb in fn.blocks:
            new_insts = []
            for inst in bb.instructions:
                si = inst.sync_info
                if si is not None and si.on_wait and len(si.on_wait) > 1:
                    waits = list(si.on_wait)
                    for w in waits[:-1]:
                        n_split += 1
                        nop = mybir.InstNoOp(
                            name=f"I-waitsplit-{n_split}",
                            engine=inst.engine,
                            ins=[],
                            outs=[],
                            sync_info=mybir.SyncInfo(on_wait=[w], on_update=[]),
                        )
                        new_insts.append(nop)
                    si.on_wait = [waits[-1]]
                new_insts.append(inst)
            bb.instructions[:] = new_insts
    return n_split


def host_prep(x, freqs_cis, mask, Wqkv, Wo):
    """Build per-core input maps + the shared schedule."""
    x = np.asarray(x, dtype=np.float32)
    freqs_cis = np.asarray(freqs_cis, dtype=np.float32)
    mask_np = np.asarray(mask).reshape(S, S).astype(bool)
    Wqkv = np.asarray(Wqkv, dtype=np.float32)
    Wo = np.asarray(Wo, dtype=np.float32)

    sched, mask_tiles, band_mode = make_schedule(mask_np)

    # p-major: xT[p, k, s] = x[s, k*128 + p]; per-partition rows contiguous
    xT = np.ascontiguousarray(
        x.reshape(S, H).T.reshape(NTILES_H, 128, S).transpose(1, 0, 2).astype(BF_NP)
    )

    cos_t = np.ascontiguousarray(freqs_cis[:, :, 0].T)  # [32, S]
    sin_t = np.ascontiguousarray(freqs_cis[:, :, 1].T)
    c64 = np.repeat(cos_t, 2, axis=0)  # [64, S]
    s64 = np.repeat(sin_t, 2, axis=0)
    ctab = np.tile(c64, (2, 1)).astype(BF_NP)  # [128, S]
    stab = np.tile(s64, (2, 1)).astype(BF_NP)

    # pswap: out[m] = -in[m+1] (m even), +in[m-1] (m odd); lhsT[k, m]
    pswap = np.zeros((128, 128), dtype=np.float32)
    for i in range(64):
        pswap[2 * i + 1, 2 * i] = -1.0
        pswap[2 * i, 2 * i + 1] = 1.0
    consts = np.zeros((128, 768), dtype=np.float32)
    consts[:, 0:128] = pswap
    consts[:, 128:256] = np.eye(128, dtype=np.float32)
    consts[:, 384:448] = 1.0
    # selector matrices: bc[m, s] = recip[32*(2*jo + m//64), s]
    for jo in range(2):
        sel = np.zeros((128, 128), dtype=np.float32)
        for m in range(128):
            sel[32 * (2 * jo + m // 64), m] = 1.0
        consts[:, 512 + 128 * jo : 640 + 128 * jo] = sel
    consts = consts.astype(BF_NP)

    band = None
    if band_mode:
        # band[tp, c] = 1.0 iff (c - 384) >= tp ; slice at 384 - (t0 - s0)
        cc = np.arange(896)[None, :] - 384
        tp = np.arange(128)[:, None]
        band = (cc >= tp).astype(BF_NP)

    in_maps = []
    for c in range(N_CORES):
        q_rows = Wqkv[c * G * HD : (c + 1) * G * HD]  # [256, H]
        k_rows = Wqkv[NH * HD + c * HD : NH * HD + (c + 1) * HD]  # [64, H]
        v_rows = Wqkv[(NH + NKV) * HD + c * HD : (NH + NKV) * HD + (c + 1) * HD]
        w_loc = np.concatenate([q_rows, k_rows, v_rows], axis=0)  # [384, H]
        # wqkvT[p, k*JL + j] = w_loc[j, k*128 + p]
        wqkvT = np.ascontiguousarray(
            w_loc.T.reshape(NTILES_H, 128, JL)
            .transpose(1, 0, 2)
            .reshape(128, NTILES_H * JL)
            .astype(BF_NP)
        )
        # woT[p, jo, o] = Wo[o, c*YL + jo*128 + p]
        woT = np.ascontiguousarray(
            Wo[:, c * YL : (c + 1) * YL]
            .T.reshape(2, 128, H)
            .transpose(1, 0, 2)
            .astype(BF_NP)
        )
        m = {
            "xT": xT,
            "wqkvT": wqkvT,
            "woT": woT,
            "ctab": ctab,
            "stab": stab,
            "consts": consts,
        }
        if band is not None:
            m["band"] = band
        if mask_tiles is not None:
            m["gmask"] = mask_tiles.astype(BF_NP)
        in_maps.append(m)

    n_gen = 0 if mask_tiles is None else mask_tiles.shape[0]
    return in_maps, sched, n_gen, band_mode


def run(x, freqs_cis, mask, Wqkv, Wo, trace=False, trace_cores=None):
    from concourse.bass_utils import run_bass_kernel_spmd

    in_maps, sched, n_gen, band_mode = host_prep(x, freqs_cis, mask, Wqkv, Wo)
    nc = build_nc(sched, n_gen, band_mode)
    res = run_bass_kernel_spmd(
        nc,
        in_maps,
        list(range(N_CORES)),
        trace=trace,
        trace_cores=trace_cores,
    )
    acc = np.zeros((H, S), dtype=np.float32)
    for c in range(N_CORES):
        acc += res.results[c]["out_t"].astype(np.float32)
    out = acc.T.astype(np.float32).reshape(1, S, H)
    return out, res


_NC_CACHE = {}


def kernel(x, freqs_cis, mask, Wqkv, Wo):
    from concourse.bass_utils import run_bass_kernel_spmd

    in_maps, sched, n_gen, band_mode = host_prep(x, freqs_cis, mask, Wqkv, Wo)
    key = (
        tuple(
            tuple(e if m is None else (e, m[0], m[1]) for e, m in es)
            for es in sched
        ),
        n_gen,
        band_mode,
    )
    if key not in _NC_CACHE:
        _NC_CACHE[key] = build_nc(sched, n_gen, band_mode)
    # transient NRT_EXEC_UNIT_UNRECOVERABLE from a previously wedged
    # device clears on retry (sometimes needs two)
    for attempt in range(3):
        try:
            res = run_bass_kernel_spmd(
                _NC_CACHE[key], in_maps, list(range(N_CORES))
            )
            break
        except Exception:
            if attempt == 2:
                raise
            import time

            time.sleep(5)
    acc = np.zeros((H, S), dtype=np.float32)
    for c in range(N_CORES):
        acc += res.results[c]["out_t"].astype(np.float32)
    return acc.T.astype(np.float32).reshape(1, S, H)
